# revision 34
# baseline (speedup 1.0000x reference)
"""Trainium2 Bass kernel for nn_BertMoELayer (B=2,S=2048,D=768,F=3072,E=8,top-2).

Strategy: expert-parallel across 8 NeuronCores (1 expert per core).
Each core receives the full token set, computes the router, selects the
tokens routed to its expert (top-2 membership), compacts their indices
on-device (sparse_gather), and runs the expert FFN in bf16.

Changes vs the v1 baseline (~320us -> ~289us):
- Gate matmuls via an fp16 hi/lo decomposition (logits ~= xh@gh + xh@gl +
  xl@gh) instead of plain fp32: fp32-level logit accuracy (max err ~2e-6,
  verified flip-free for the top-2 selection) at 16-bit PE rate.
- FFN datapath in bf16, all weights resident in SBUF (v1 re-streamed the
  9.4MB w_down from HBM on every one of the 3 chunks).  Note: on real TRN2
  both bf16 and fp16 matmul run at 2 cycles/row (the cost model's 1 cy/row
  is not what the silicon does); fp16 additionally hits the fp32-style
  two-pass path in some shapes, so bf16 is the right 16-bit choice.
- A dummy sparse_gather at t=0 pre-loads the sparse_gather ucode library
  off the critical path; the combine-weight sparse_gather is deferred past
  the token gathers so only one lib switch (to the mlp library for
  dma_gather/dma_scatter_add) sits on the routing critical path.
- idx replication for the ucode gathers/scatters via a single PE broadcast
  matmul (v1 used 7 serial gpsimd SBUF copies); combine weights reach the
  [128, 9] slot-major layout via a small DRAM bounce instead of v1's
  two-transpose round trip.
- Compaction outputs are pre-filled with sentinels so the tail beyond
  num_found is always safe; output is [XPAD, D] and junk rows are dropped
  on the host.

Self-contained: hardcodes all shapes; only imports the installed concourse
stack from /opt/trn_rl_repo.
"""
import sys

sys.path.insert(0, "/opt/trn_rl_repo")

import numpy as np

import concourse.bass as bass
import concourse.tile as tile
from concourse import bacc, mybir
from concourse.bass import ds, ts, IndirectOffsetOnAxis
from concourse.bass_utils import run_bass_kernel_spmd

# Problem shapes
B, S, D, F, E = 2, 2048, 768, 3072, 8
T = B * S                 # 4096 tokens
CAP = 1152                # per-expert slot capacity
XPAD = T + 128            # x_pad rows; row T is the junk/sentinel row
DC = D // 128             # 6 contraction chunks for up-proj
FC = F // 128             # 24 F tiles
NT = T // 128             # 32 token tiles
NCH = 3                   # FFN slot chunks
CHS = CAP // NCH          # 384 slots per chunk
NG = 8                    # gate groups of 512 tokens
SENT_N = 256              # sentinel candidates appended after real tokens
CAND_F = (T + SENT_N) // 16  # 272 candidate free-dim
SENT_F = T // 16          # 256: sentinel region starts here
SLOTC = CAP // 128        # 9 slot columns in slot-major layout
HW_ = D // 2              # 384: down-proj half width

F32 = mybir.dt.float32
F32R = mybir.dt.float32r
F16 = mybir.dt.float16
BF16 = mybir.dt.bfloat16
I32 = mybir.dt.int32
U32 = mybir.dt.uint32
ALU = mybir.AluOpType
AXX = mybir.AxisListType
ACT = mybir.ActivationFunctionType

GATE_DT = F32             # gate matmul dtype (F32 exact; F32R is 4x faster)


def build_program():
    nc = bacc.Bacc("TRN2", target_bir_lowering=False, debug=False)

    x_pad = nc.dram_tensor("x_pad", (XPAD, D), BF16, kind="ExternalInput")
    xth = nc.dram_tensor("xth", (D, T), F16, kind="ExternalInput")
    xtl = nc.dram_tensor("xtl", (D, T), F16, kind="ExternalInput")
    gwhl = nc.dram_tensor("gwhl", (D, 2 * E), F16, kind="ExternalInput")
    stack2 = nc.dram_tensor("stack2", (2 * E, E), F32, kind="ExternalInput")
    bcast16 = nc.dram_tensor("bcast16", (16, 128), F32, kind="ExternalInput")
    wup = nc.dram_tensor("wup", (D, F), BF16, kind="ExternalInput")
    bup = nc.dram_tensor("bup", (F,), F32, kind="ExternalInput")
    # wdn_r[half*FC + m] = w_down[m*128:(m+1)*128, half*384:(half+1)*384]
    wdn_r = nc.dram_tensor("wdn_r", (2 * FC, 128, HW_), BF16,
                           kind="ExternalInput")
    bdn = nc.dram_tensor("bdn", (D,), BF16, kind="ExternalInput")
    ids = nc.dram_tensor("ids", (128, NT), F32, kind="ExternalInput")
    ident = nc.dram_tensor("ident", (128, 128), GATE_DT, kind="ExternalInput")
    ident16 = nc.dram_tensor("ident16", (128, 128), BF16, kind="ExternalInput")
    ones16 = nc.dram_tensor("ones16", (1, 128), BF16, kind="ExternalInput")
    out = nc.dram_tensor("out", (XPAD, D), F32, kind="ExternalOutput")

    with tile.TileContext(nc) as tc:
        with (
            tc.tile_pool(name="const", bufs=1) as const_pool,
            tc.tile_pool(name="dram", bufs=1, space="DRAM") as dram_pool,
            tc.tile_pool(name="route", bufs=1) as route_pool,
        ):
            # ---- constants / small inputs ----
            # gwhl first on the sync queue so the gate matmuls can start as
            # soon as group 0 lands; small consts go via the scalar queue
            gwhl_sb = const_pool.tile([128, DC, 2 * E], F16)
            nc.sync.dma_start(gwhl_sb[:],
                              gwhl.rearrange("(kc p) e -> p kc e", p=128))
            ident_sb = const_pool.tile([128, 128], GATE_DT)
            nc.scalar.dma_start(ident_sb[:], ident[:])
            ident16_sb = const_pool.tile([128, 128], BF16)
            nc.scalar.dma_start(ident16_sb[:], ident16[:])
            stack2_sb = const_pool.tile([2 * E, E], F32)
            nc.scalar.dma_start(stack2_sb[:], stack2[:])
            bcast_sb = const_pool.tile([16, 128], F32)
            nc.scalar.dma_start(bcast_sb[:], bcast16[:])
            ids_sb = const_pool.tile([128, NT], F32)
            nc.scalar.dma_start(ids_sb[:], ids[:])
            bup_sb = const_pool.tile([128, FC], F32)
            nc.scalar.dma_start(bup_sb[:], bup.rearrange("(m p) -> p m", p=128))
            bdn_sb = const_pool.tile([1, D], BF16)
            nc.scalar.dma_start(bdn_sb[:], bdn[None, :])
            ones_sb = const_pool.tile([1, 128], BF16)
            nc.scalar.dma_start(ones_sb[:], ones16[:])

            # dummy sparse_gather so the ucode library loads at t=0 (the only
            # gpsimd ucode op in the program; indirect DMAs need no library)
            dummy_in = const_pool.tile([16, 16], F32)
            nc.any.memset(dummy_in[:], 1.0)
            dummy_out = const_pool.tile([16, 16], F32)
            dummy_nf = const_pool.tile([1, 1], U32)
            nc.gpsimd.sparse_gather(dummy_out[:], dummy_in[:],
                                    num_found=dummy_nf[:])

            # ---- resident FFN weights (loaded after the gate xT loads) ----
            wup_a = const_pool.tile([128, DC, F // 2], BF16)
            wup_b = const_pool.tile([128, DC, F // 2], BF16)
            wdn_a = const_pool.tile([128, FC, HW_], BF16)
            wdn_b = const_pool.tile([128, FC, HW_], BF16)

            # ---- routing products (survive into the FFN phase) ----
            sgid16 = route_pool.tile([16, CAND_F], F32)
            sgcw16 = route_pool.tile([16, CAND_F], F32)
            idx16 = route_pool.tile([128, CAP // 16], mybir.dt.int16)
            cw_sl = route_pool.tile([128, SLOTC], F32)
            xg_c = [route_pool.tile([128, NCH, D], BF16, name=f"xg{c}",
                                    tag=f"xg{c}")
                    for c in range(NCH)]
            scr_cw = dram_pool.tile([CAP], F32, tag="scr_cw")

            # =========== GATE PHASE ===========
            with (
                tc.tile_pool(name="gxt", bufs=4) as gxt_pool,
                tc.tile_pool(name="glt", bufs=2) as glt_pool,
                tc.tile_pool(name="gsoft", bufs=1) as gsoft_pool,
                tc.tile_pool(name="gps_tr", bufs=2, space="PSUM") as gps_tr,
                tc.tile_pool(name="gps_lt", bufs=2, space="PSUM") as gps_lt,
                tc.tile_pool(name="gps_ln", bufs=2, space="PSUM") as gps_ln,
            ):
                # fp32-accurate gate via fp16 hi/lo decomposition:
                # logits ~= xh@gh + xh@gl + xl@gh, all at bf16 PE rate.
                # lps rows 0:8 accumulate gh^T xh + gh^T xl; rows 8:16 gl^T xh;
                # the transpose matmul vs stack2=[I8;I8] sums the halves.
                logits_sb = gsoft_pool.tile([128, NT, E], F32)
                for g in range(NG):
                    xTh_g = gxt_pool.tile([128, DC, 512], F16, tag="xTh")
                    xTl_g = gxt_pool.tile([128, DC, 512], F16, tag="xTl")
                    src_h = xth[:, g * 512:(g + 1) * 512].rearrange(
                        "(kc p) t -> p kc t", p=128
                    )
                    src_l = xtl[:, g * 512:(g + 1) * 512].rearrange(
                        "(kc p) t -> p kc t", p=128
                    )
                    if g == 0:
                        # split group 0 by kc so the first matmuls start
                        # after ~0.5MB instead of the full 1.57MB
                        nc.sync.dma_start(xTh_g[:, 0:2, :], src_h[:, 0:2, :])
                        nc.sync.dma_start(xTl_g[:, 0:2, :], src_l[:, 0:2, :])
                        nc.sync.dma_start(xTh_g[:, 2:DC, :], src_h[:, 2:DC, :])
                        nc.sync.dma_start(xTl_g[:, 2:DC, :], src_l[:, 2:DC, :])
                    else:
                        nc.sync.dma_start(xTh_g[:], src_h)
                        nc.sync.dma_start(xTl_g[:], src_l)
                    lps = gps_lt.tile([2 * E, 512], F32, tag="lt")
                    for kc in range(DC):
                        if kc == DC - 1:
                            nc.tensor.matmul(
                                lps[0:E, :], gwhl_sb[:, kc, 0:E],
                                xTl_g[:, kc, :], start=False, stop=False,
                            )
                            nc.tensor.matmul(
                                lps[:], gwhl_sb[:, kc, :], xTh_g[:, kc, :],
                                start=False, stop=True,
                            )
                        else:
                            nc.tensor.matmul(
                                lps[:], gwhl_sb[:, kc, :], xTh_g[:, kc, :],
                                start=(kc == 0), stop=False,
                            )
                            nc.tensor.matmul(
                                lps[0:E, :], gwhl_sb[:, kc, 0:E],
                                xTl_g[:, kc, :], start=False, stop=False,
                            )
                    lT_sb = glt_pool.tile([2 * E, 512], F32, tag="lT")
                    nc.any.tensor_copy(lT_sb[:], lps[:])
                    for j in range(4):
                        t = g * 4 + j
                        pn = gps_ln.tile([128, 8], F32, tag="ln")
                        nc.tensor.matmul(
                            pn[:], lT_sb[:, ts(j, 128)], stack2_sb[:]
                        )
                        nc.any.tensor_copy(logits_sb[:, t, :], pn[:])

                # resident weight loads, queued behind the gate xT stream
                nc.sync.dma_start(
                    wup_a[:],
                    wup.rearrange("(kc p) f -> p kc f", p=128)[:, :, 0:F // 2],
                )
                nc.sync.dma_start(
                    wup_b[:],
                    wup.rearrange("(kc p) f -> p kc f", p=128)[:, :, F // 2:F],
                )
                nc.sync.dma_start(
                    wdn_a[:], wdn_r[0:FC, :, :].rearrange("t p h -> p t h")
                )
                nc.sync.dma_start(
                    wdn_b[:], wdn_r[FC:2 * FC, :, :].rearrange("t p h -> p t h")
                )

                # ---- batched softmax + top-2 over all 32 tiles ----
                m1 = gsoft_pool.tile([128, NT], F32)
                nc.vector.tensor_reduce(m1[:], logits_sb[:], AXX.X, ALU.max)
                smx = gsoft_pool.tile([128, NT, E], F32)
                for e in range(E):
                    nc.vector.tensor_sub(
                        smx[:, :, e], logits_sb[:, :, e], m1[:]
                    )
                nc.scalar.activation(
                    smx[:].rearrange("p a b -> p (a b)"),
                    smx[:].rearrange("p a b -> p (a b)"), ACT.Exp,
                )
                zsum = gsoft_pool.tile([128, NT], F32)
                nc.vector.tensor_reduce(zsum[:], smx[:], AXX.X, ALU.add)
                rz = gsoft_pool.tile([128, NT], F32)
                nc.vector.reciprocal(rz[:], zsum[:])
                gt8 = gsoft_pool.tile([128, NT, E], F32)
                for e in range(E):
                    nc.vector.tensor_tensor(
                        gt8[:, :, e], logits_sb[:, :, e], logits_sb[:, :, 0],
                        op=ALU.is_gt,
                    )
                cnt = gsoft_pool.tile([128, NT], F32)
                nc.vector.tensor_reduce(cnt[:], gt8[:], AXX.X, ALU.add)
                mask = gsoft_pool.tile([128, NT], F32)
                nc.vector.tensor_scalar(mask[:], cnt[:], 1.5, None, op0=ALU.is_lt)
                mm1 = gsoft_pool.tile([128, NT], F32)
                nc.vector.tensor_scalar_add(mm1[:], mask[:], -1.0)
                cw0 = gsoft_pool.tile([128, NT], F32)
                nc.vector.tensor_tensor(cw0[:], smx[:, :, 0], rz[:], op=ALU.mult)

                # candidates: token id / combine weight if selected else -1
                cand_id = gsoft_pool.tile([128, NT], F32)
                cand_cw = gsoft_pool.tile([128, NT], F32)
                nc.vector.tensor_tensor(cand_cw[:], cw0[:], mask[:], op=ALU.mult)
                nc.vector.tensor_add(cand_cw[:], cand_cw[:], mm1[:])
                nc.vector.tensor_tensor(cand_id[:], ids_sb[:], mask[:],
                                        op=ALU.mult)
                nc.vector.tensor_add(cand_id[:], cand_id[:], mm1[:])

                # ---- compaction ----
                # regroup [128,32] -> [16,256] via PE transpose (any candidate
                # order works; only "sentinels last" matters)
                cand16_id = gsoft_pool.tile([16, CAND_F], F32)
                cand16_cw = route_pool.tile([16, CAND_F], F32)
                for cbuf, c16 in ((cand_id, cand16_id), (cand_cw, cand16_cw)):
                    pct = gps_tr.tile([32, 128], F32, tag="tr")
                    nc.tensor.matmul(pct[:], cbuf[:], ident_sb[:])
                    ctT = gsoft_pool.tile([32, 128], F32, tag="ctT")
                    nc.any.tensor_copy(ctT[:], pct[:])
                    nc.vector.tensor_copy(c16[:, 0:128], ctT[0:16, :])
                    nc.gpsimd.dma_start(c16[:, 128:256], ctT[16:32, :])
                # sentinel candidates: token T (junk row), weight 0
                nc.any.memset(cand16_id[:, SENT_F:CAND_F], float(T))
                nc.any.memset(cand16_cw[:, SENT_F:CAND_F], 0.0)

                # pre-fill the compaction outputs so the tail beyond
                # num_found is safe even under extreme expert load
                nc.any.memset(sgid16[:], float(T))
                nc.any.memset(sgcw16[:], 0.0)
                nf1 = route_pool.tile([1, 1], U32)
                nc.gpsimd.sparse_gather(sgid16[:], cand16_id[:],
                                        num_found=nf1[:])

            # =========== FFN PHASE ===========
            with (
                tc.tile_pool(name="fmisc", bufs=1) as fmisc_pool,
                tc.tile_pool(name="fxt", bufs=2) as fxt_pool,
                tc.tile_pool(name="fh", bufs=2) as fh_pool,
                tc.tile_pool(name="fy", bufs=2) as fy_pool,
                tc.tile_pool(name="fps_tr", bufs=2, space="PSUM") as fps_tr,
                tc.tile_pool(name="fps_up", bufs=2, space="PSUM") as fps_up,
                tc.tile_pool(name="fps_dn", bufs=4, space="PSUM") as fps_dn,
            ):
                # int16 idx, 16-wrapped, replicated to all 8 q7 groups via a
                # single PE broadcast matmul (no gpsimd, no DMA round-trip)
                pbi = fps_tr.tile([128, CAP // 16], F32, tag="tr")
                nc.tensor.matmul(pbi[:], bcast_sb[:], sgid16[:, 0:CAP // 16])
                nc.vector.tensor_copy(idx16[:], pbi[:])

                # token-row gathers (gpsimd ucode dma_gather, mlp library);
                # chunk 0 split into 128-slot pieces so each j-block's
                # transposes overlap the next piece's transfer
                for j in range(NCH):
                    nc.gpsimd.dma_gather(
                        xg_c[0][:, j:j + 1, :], x_pad[:],
                        idx16[:, 8 * j:8 * j + 8],
                        num_idxs=128, num_idxs_reg=128, elem_size=D,
                    )
                for c in range(1, NCH):
                    nc.gpsimd.dma_gather(
                        xg_c[c][:], x_pad[:],
                        idx16[:, c * (CHS // 16):(c + 1) * (CHS // 16)],
                        num_idxs=CHS, num_idxs_reg=CHS, elem_size=D,
                    )
                # combine-weight compaction off the critical path (its own
                # ucode library; runs while chunk 0 computes)
                nf2 = route_pool.tile([1, 1], U32)
                nc.gpsimd.sparse_gather(sgcw16[:], cand16_cw[:],
                                        num_found=nf2[:])
                # cw: [16,272] wrapped stream -> DRAM -> [128,9] slot-major
                nc.scalar.dma_start(
                    scr_cw[:].rearrange("(f b) -> b f", b=16),
                    sgcw16[:, 0:CAP // 16],
                )
                nc.scalar.dma_start(
                    cw_sl[:], scr_cw[:].rearrange("(j p) -> p j", p=128)
                )

                for c in range(NCH):
                    # transpose gathered rows: [slot, D] -> [D, slot]
                    xcT = fxt_pool.tile([128, DC, CHS], BF16, tag="xcT")
                    for j in range(NCH):
                        for kc in range(DC):
                            pt = fps_tr.tile([128, 128], F32, tag="tr")
                            nc.tensor.matmul(
                                pt[:], xg_c[c][:, j, ts(kc, 128)],
                                ident16_sb[:]
                            )
                            nc.any.tensor_copy(
                                xcT[:, kc, ds(j * 128, 128)], pt[:]
                            )
                    # up-projection + gelu -> h^T [128, FC, CHS] fp16
                    h_sb = fh_pool.tile([128, FC, CHS], BF16, tag="h")
                    for m in range(FC):
                        wtile = wup_a if m < FC // 2 else wup_b
                        ml = m if m < FC // 2 else m - FC // 2
                        psu = fps_up.tile([128, CHS], F32, tag="up")
                        for kc in range(DC):
                            nc.tensor.matmul(
                                psu[:],
                                wtile[:, kc, ts(ml, 128)],
                                xcT[:, kc, :],
                                start=(kc == 0), stop=(kc == DC - 1),
                            )
                        nc.scalar.activation(
                            h_sb[:, m, :], psu[:], ACT.Gelu,
                            bias=bup_sb[:, m:m + 1],
                        )
                    # down-projection + bias + combine scale, one 128-slot
                    # block at a time so each block's rows scatter as soon as
                    # they are complete (shrinks the end-of-kernel DMA drain)
                    y = fy_pool.tile([128, NCH, D], F32, tag="y")
                    ccol = c * (CHS // 16)
                    for blk in range(NCH):
                        psd0 = fps_dn.tile([128, HW_], F32, tag="dn",
                                           name=f"psd{c}_{blk}_0")
                        psd1 = fps_dn.tile([128, HW_], F32, tag="dn",
                                           name=f"psd{c}_{blk}_1")
                        for m in range(FC):
                            nc.tensor.matmul(
                                psd0[:], h_sb[:, m, ts(blk, 128)],
                                wdn_a[:, m, :], start=(m == 0), stop=False,
                            )
                            nc.tensor.matmul(
                                psd1[:], h_sb[:, m, ts(blk, 128)],
                                wdn_b[:, m, :], start=(m == 0), stop=False,
                            )
                        for half, psd in ((0, psd0), (1, psd1)):
                            nc.tensor.matmul(
                                psd[:],
                                ones_sb[0:1, 0:128],
                                bdn_sb[0:1, ds(half * HW_, HW_)],
                                start=False, stop=True,
                            )
                            nc.vector.tensor_scalar(
                                y[:, blk, ds(half * HW_, HW_)],
                                psd[:],
                                cw_sl[:, NCH * c + blk:NCH * c + blk + 1],
                                None,
                                op0=ALU.mult,
                            )
                        # scatter-add this block's rows into the padded
                        # output; sentinel slots land in the junk rows >= T
                        nc.gpsimd.dma_scatter_add(
                            out[:], y[:, blk:blk + 1, :],
                            idx16[:, ccol + 8 * blk:ccol + 8 * blk + 8],
                            num_idxs=128, num_idxs_reg=128, elem_size=D,
                        )

    nc.finalize()
    return nc


_NC_CACHE = None


def _get_program():
    global _NC_CACHE
    if _NC_CACHE is None:
        _NC_CACHE = build_program()
    return _NC_CACHE


def make_in_maps(hidden_states, gate_w, w_up, b_up, w_down, b_down):
    hidden_states = np.asarray(hidden_states, dtype=np.float32)
    gate_w = np.asarray(gate_w, dtype=np.float32)
    w_up = np.asarray(w_up, dtype=np.float32)
    b_up = np.asarray(b_up, dtype=np.float32)
    w_down = np.asarray(w_down, dtype=np.float32)
    b_down = np.asarray(b_down, dtype=np.float32)

    import ml_dtypes
    bf16 = ml_dtypes.bfloat16
    x = hidden_states.reshape(T, D)
    x_pad = np.zeros((XPAD, D), dtype=bf16)
    x_pad[:T] = x.astype(bf16)
    xT_host = np.ascontiguousarray(x.T)
    xth = xT_host.astype(np.float16)
    xtl = (xT_host - xth.astype(np.float32)).astype(np.float16)
    ids = np.arange(T, dtype=np.float32).reshape(NT, 128).T.copy()  # [128, NT]
    ident = np.eye(128, dtype=np.float32)
    ident16 = np.eye(128, dtype=bf16)
    eye8 = np.eye(E, dtype=np.float32)
    stack2 = np.concatenate([eye8, eye8], axis=0)  # [16, 8]
    bcast16 = np.tile(np.eye(16, dtype=np.float32), (1, 8))  # [16, 128]

    in_maps = []
    for c in range(E):
        gwc = np.concatenate([gate_w[:, c:], gate_w[:, :c]], axis=1)
        gh = gwc.astype(np.float16)
        gl = (gwc - gh.astype(np.float32)).astype(np.float16)
        gwhl = np.concatenate([gh, gl], axis=1)  # [D, 16]
        wdn = w_down[c]  # [F, D]
        wdn_r = np.ascontiguousarray(
            wdn.reshape(FC, 128, 2, HW_).transpose(2, 0, 1, 3)
        ).reshape(2 * FC, 128, HW_).astype(bf16)
        in_maps.append({
            "x_pad": x_pad,
            "xth": xth,
            "xtl": xtl,
            "gwhl": gwhl,
            "stack2": stack2,
            "bcast16": bcast16,
            "wup": np.ascontiguousarray(w_up[c]).astype(bf16),
            "bup": np.ascontiguousarray(b_up[c]),
            "wdn_r": wdn_r,
            "bdn": np.ascontiguousarray(b_down[c]).astype(bf16),
            "ids": ids,
            "ident": ident,
            "ident16": ident16,
            "ones16": np.ones((1, 128), dtype=bf16),
        })
    return in_maps


def combine_results(results):
    out = np.zeros((T, D), dtype=np.float32)
    for c in range(E):
        out += results[c]["out"][:T]
    return out.reshape(B, S, D)


def kernel(hidden_states, gate_w, w_up, b_up, w_down, b_down):
    in_maps = make_in_maps(hidden_states, gate_w, w_up, b_up, w_down, b_down)
    nc = _get_program()
    res = run_bass_kernel_spmd(nc, in_maps, core_ids=list(range(E)))
    return combine_results(res.results)


if __name__ == "__main__":
    rng = np.random.default_rng(0)
    hs = rng.standard_normal((B, S, D)).astype(np.float32)
    gw = rng.standard_normal((D, E)).astype(np.float32) / np.sqrt(D)
    wu = (rng.standard_normal((E, D, F)) * 0.02).astype(np.float32)
    bu = np.zeros((E, F), dtype=np.float32)
    wd = (rng.standard_normal((E, F, D)) * 0.02).astype(np.float32)
    bd = np.zeros((E, D), dtype=np.float32)
    out = kernel(hs, gw, wu, bu, wd, bd)
    print("out", out.shape, out.dtype, np.abs(out).max())
